# revision 35
# baseline (speedup 1.0000x reference)
"""Multi-head "channel attention" kernel for Trainium2 (8 NeuronCores).

Reference computation (B=16, D=512, N=2048, h=8 heads, Nh=256):
    q = Wq @ XQ ; k = Wk @ XK ; v = Wv @ XV          (per batch, (D,N))
    per head (N split into 8 chunks of 256):
      scores = q_h @ k_h^T / sqrt(Nh)                ((D,D), contract over Nh)
      p      = softmax(scores, axis=-1)
      o_h    = p @ v_h                               ((D,Nh), contract over D)
    attn = concat(o_h) ; out = Wo @ (XQ - attn)
Sharding: data-parallel over batch: 16 batches / 8 cores = 2 per core.

fp8 strategy (rel err ~7e-3 vs the 2e-2 gate, validated by emulation):
  * Every matmul except the final output projection runs in fp8e4m3 with
    MatmulPerfMode.DoubleRow: 256 contraction rows per instruction at
    1 cycle per output-free element = 2x the fp32r/bf16 rate.
  * The host ships XK/XV/XQ quantized to fp8 (plus a bf16 copy of XQ for
    the residual add), and Wq/Wk/Wv transposed and scaled by 16 in fp8
    (unscaled they'd sit in the subnormal range), Wo transposed in bf16.
  * The x16 weight scales cancel: exp(scale*x) uses scale 1/(256*16);
    V's guard columns hold -16 so PSUM accumulates -16*r, and
    Z = XQ + O * (-1/(16 r)) == XQ - attn.
  * exp also carries bias -ln(16), attenuating p~ = exp(s)/16 into
    e4m3's range (max |p| ~42 vs the 240 cap; scores are ~N(0,1) but the
    tail over 4M samples reaches ~6.5).
  * The attention term is only ~7% of Z's magnitude, so fp8 noise in the
    whole attention pipeline is diluted ~14x; the bf16 output projection
    sets the ~2e-3 floor.
  * Heads are processed in PAIRS (adjacent heads = adjacent n-columns),
    making most matmul frees and all drains 512 wide, halving fixed
    per-instruction costs. O-matmul free is 260 (odd frees like 258 run
    ~8x slow on the PE; 260 is even and full speed).
  * PSUM->SBUF drains are the co-bottleneck (DMA and GPSIMD have no PSUM
    port): split between ScalarE (exp, o, kt-half) and VectorE (qt, v,
    kt-half, STT, recip), each ~85us vs ~97us of PE work.
"""

import sys

if "/opt/trn_rl_repo" not in sys.path:
    sys.path.insert(0, "/opt/trn_rl_repo")

import numpy as np
import ml_dtypes

import concourse.bass as bass
import concourse.tile as tile
from concourse import bacc, mybir
from concourse.bass_utils import run_bass_kernel_spmd

B_PER_CORE = 2
D = 512
N = 2048
H = 8
NH = N // H  # 256
PT = D // 128  # 4 partition tiles over D
HP = H // 2  # 4 head pairs
NP = 2 * NH  # 512 columns per head pair

F32 = mybir.dt.float32
F8 = mybir.dt.float8e4
BF16 = mybir.dt.bfloat16
DR = mybir.MatmulPerfMode.DoubleRow

NPF8 = ml_dtypes.float8_e4m3
NPBF16 = ml_dtypes.bfloat16

WS = 16.0  # host-side weight scale for Wq/Wk/Wv in fp8
EXP_SCALE = 1.0 / (WS * WS * np.sqrt(NH))  # = 1/4096
EXP_BIAS = float(-np.log(16.0))  # p~ = exp(s)/16
VG = 260  # per-head V row: 256 data + 2 guard (-16) + 2 pad

_NC_CACHE = None


def build_nc():
    nc = bacc.Bacc("TRN2", target_bir_lowering=False, debug=False)

    xq8 = nc.dram_tensor("xq8", [B_PER_CORE, D, N], F8, kind="ExternalInput").ap()
    xqb = nc.dram_tensor("xqb", [B_PER_CORE, D, N], BF16, kind="ExternalInput").ap()
    xk8 = nc.dram_tensor("xk8", [B_PER_CORE, D, N], F8, kind="ExternalInput").ap()
    xv8 = nc.dram_tensor("xv8", [B_PER_CORE, D, N], F8, kind="ExternalInput").ap()
    wq8 = nc.dram_tensor("wq8", [D, D], F8, kind="ExternalInput").ap()
    wk8 = nc.dram_tensor("wk8", [D, D], F8, kind="ExternalInput").ap()
    wv8 = nc.dram_tensor("wv8", [D, D], F8, kind="ExternalInput").ap()
    wob = nc.dram_tensor("wob", [D, D], BF16, kind="ExternalInput").ap()
    out = nc.dram_tensor("out", [B_PER_CORE, D, N], BF16, kind="ExternalOutput").ap()

    with tile.TileContext(nc) as tc:
        with (
            tc.tile_pool(name="wpool", bufs=1) as wpool,
            tc.tile_pool(name="xpool", bufs=1) as xpool,
            tc.tile_pool(name="qkpool", bufs=6) as qkpool,
            tc.tile_pool(name="vpool", bufs=4) as vpool,
            tc.tile_pool(name="ptpool", bufs=6) as ptpool,
            tc.tile_pool(name="zpool", bufs=3) as zpool,
            tc.tile_pool(name="opool", bufs=6) as opool,
            tc.tile_pool(name="rpool", bufs=8) as rpool,
            tc.tile_pool(name="ps1", bufs=4, space="PSUM") as ps1,
            tc.tile_pool(name="psS", bufs=1, space="PSUM") as psS,
            tc.tile_pool(name="pso", bufs=1, space="PSUM") as pso,
        ):
            # Resident weights: [p, it, o] = W.T[it*128+p, o] (fp8 x16),
            # loaded in per-it chunks to shorten the first matmul's dep.
            w_sb = {
                "wq": wpool.tile([128, PT, D], F8, name="w_wq", tag="w_wq"),
                "wk": wpool.tile([128, PT, D], F8, name="w_wk", tag="w_wk"),
                "wv": wpool.tile([128, PT, D], F8, name="w_wv", tag="w_wv"),
                "wo": wpool.tile([128, PT, D], BF16, name="w_wo", tag="w_wo"),
            }
            w_dram = {"wq": wq8, "wk": wk8, "wv": wv8, "wo": wob}

            def load_w(name):
                src = w_dram[name].rearrange("(t p) o -> p t o", p=128)
                nc.sync.dma_start(out=w_sb[name], in_=src)

            x_b = {
                "xq8": [xq8[b].rearrange("(t p) n -> p t n", p=128) for b in range(B_PER_CORE)],
                "xqb": [xqb[b].rearrange("(t p) n -> p t n", p=128) for b in range(B_PER_CORE)],
                "xk8": [xk8[b].rearrange("(t p) n -> p t n", p=128) for b in range(B_PER_CORE)],
                "xv8": [xv8[b].rearrange("(t p) n -> p t n", p=128) for b in range(B_PER_CORE)],
            }
            x_dt = {"xq8": F8, "xqb": BF16, "xk8": F8, "xv8": F8}

            # Whole-batch input tiles, DMA'd in per-it chunks (256-512 KiB,
            # 2 KiB bursts) up front; per-pair slices view into these.
            x_sb = [
                {
                    nm: xpool.tile(
                        [128, PT, N], x_dt[nm], name=f"{nm}_b{b}", tag=f"{nm}_b{b}"
                    )
                    for nm in ("xq8", "xqb", "xk8", "xv8")
                }
                for b in range(B_PER_CORE)
            ]

            def load_x(b, nm):
                nc.sync.dma_start(out=x_sb[b][nm], in_=x_b[nm][b])

            steps = [(b, hp) for b in range(B_PER_CORE) for hp in range(HP)]
            NSTEP = len(steps)
            # per-step live tiles for the software pipeline
            st_qt = {}
            st_kt = {}
            st_v = {}
            st_pt = {}
            # (b, hp, z_pair) whose output projection hasn't been emitted yet
            pending_out = []

            # Output DMAs are merged (all four dt groups per dma_start) to
            # cut the ~600ns/DMA sync-engine descriptor-gen cost.
            o_pair = {}

            def emit_outproj_group(b, hp, z_p, dt_, eng="act"):
                """One [128, 512] output-projection group (bf16)."""
                out_b = out[b].rearrange("(t p) n -> p t n", p=128)
                ns_ = slice(hp * NP, (hp + 1) * NP)
                ps = ps1.tile([128, D], F32, name="ps_p", tag="ps_p")
                for it in range(PT):
                    nc.tensor.matmul(
                        ps,
                        lhsT=w_sb["wo"][:, it, dt_ * 128 : (dt_ + 1) * 128],
                        rhs=z_p[:, it, :],
                        start=(it == 0),
                        stop=(it == PT - 1),
                    )
                if dt_ == 0:
                    o_pair[(b, hp)] = opool.tile([128, PT, D], BF16, name="o_sb", tag="o_sb")
                o_sb = o_pair[(b, hp)]
                if eng == "act":
                    nc.scalar.copy(out=o_sb[:, dt_, :], in_=ps)
                else:
                    nc.vector.tensor_copy(out=o_sb[:, dt_, :], in_=ps)
                if dt_ == PT - 1:
                    nc.sync.dma_start(
                        out=out_b[:, :, ns_], in_=o_pair.pop((b, hp))
                    )

            warm = wpool.tile([128, D], BF16, name="warm", tag="warm")
            bias_t = wpool.tile([128, 1], F32, name="bias_t", tag="bias_t")

            def xs(idx):
                b, hp = steps[idx]
                ns = slice(hp * NP, (hp + 1) * NP)
                return {nm: t[:, :, ns] for nm, t in x_sb[b].items()}

            # ps1 drains alternate between ScalarE and VectorE per group.
            dr_tog = [0]

            def drain(dst, src):
                dr_tog[0] ^= 1
                if dr_tog[0]:
                    nc.scalar.copy(out=dst, in_=src)
                else:
                    nc.vector.tensor_copy(out=dst, in_=src)

            def gen_s1s2(idx):
                """12 psum-groups: QT/KT (8) then V (4), all fp8 DR."""
                xt = xs(idx)
                st_qt[idx] = []
                st_kt[idx] = []
                for hl in range(2):
                    qt_h = qkpool.tile([128, 2, D], F8, name="qt_h", tag="qt_h")
                    kt_h = qkpool.tile([128, 2, D], F8, name="kt_h", tag="kt_h")
                    for dst, src, w in ((qt_h, xt["xq8"], "wq"), (kt_h, xt["xk8"], "wk")):
                        for jt in range(2):
                            nb = hl * NH + jt * 128
                            ps = ps1.tile([128, D], F32, name="ps_p", tag="ps_p")
                            for itp in range(2):
                                nc.tensor.matmul(
                                    ps,
                                    lhsT=src[:, 2 * itp : 2 * itp + 2, nb : nb + 128],
                                    rhs=w_sb[w][:, 2 * itp : 2 * itp + 2, :],
                                    start=(itp == 0),
                                    stop=(itp == 1),
                                    perf_mode=DR,
                                )
                            drain(dst[:, jt, :], ps)
                            yield
                    st_qt[idx].append(qt_h)
                    st_kt[idx].append(kt_h)
                v_p = vpool.tile([128, PT, 2 * VG], F8, name="v_p", tag="v_p")
                st_v[idx] = v_p
                guard = v_p.rearrange("p a (g c) -> p (a g) c", c=VG)[:, :, NH:VG]
                gw = VG - NH
                nc.scalar.activation(
                    out=guard,
                    in_=w_sb["wv"][:, 0, 0 : 8 * gw].rearrange("p (a c) -> p a c", c=gw),
                    func=mybir.ActivationFunctionType.Copy,
                    bias=-16.0,
                    scale=0.0,
                )
                for et in range(PT):
                    ps = ps1.tile([128, D], F32, name="ps_p", tag="ps_p")
                    for itp in range(2):
                        nc.tensor.matmul(
                            ps,
                            lhsT=w_sb["wv"][:, 2 * itp : 2 * itp + 2, et * 128 : (et + 1) * 128],
                            rhs=xt["xv8"][:, 2 * itp : 2 * itp + 2, :],
                            start=(itp == 0),
                            stop=(itp == 1),
                            perf_mode=DR,
                        )
                    dst = v_p[:, et, :].rearrange("p (g c) -> p g c", c=VG)[:, :, 0:NH]
                    drain(dst, ps.rearrange("p (g c) -> p g c", c=NH))
                    yield

            def gen_s3(idx):
                """4 psum-groups: scores^T into 2-bank tiles, wide exp."""
                qt, kt = st_qt.pop(idx), st_kt.pop(idx)
                st_pt[idx] = []
                for hl in range(2):
                    pt_h = ptpool.tile([128, PT, D], F8, name="pt_h", tag="pt_h")
                    for ep in range(2):
                        ps = psS.tile([128, 2, D], F32, name="ps_s", tag="ps_s")
                        for i in range(2):
                            et = 2 * ep + i
                            nc.tensor.matmul(
                                ps[:, i, :],
                                lhsT=kt[hl][:, 0:2, et * 128 : (et + 1) * 128],
                                rhs=qt[hl][:, 0:2, :],
                                start=True,
                                stop=True,
                                perf_mode=DR,
                            )
                        nc.scalar.activation(
                            out=pt_h[:, 2 * ep : 2 * ep + 2, :],
                            in_=ps,
                            func=mybir.ActivationFunctionType.Exp,
                            scale=float(EXP_SCALE),
                            bias=bias_t,
                        )
                        yield
                    st_pt[idx].append(pt_h)

            def gen_s4(idx):
                """4 psum-groups: O + recip + STT; consumes one pending
                output-projection group after each."""
                b, hp = steps[idx]
                xt = xs(idx)
                pt = st_pt.pop(idx)
                v_p = st_v.pop(idx)
                z_p = zpool.tile([128, PT, NP], BF16, name="z_p", tag="z_p")
                for hl in range(2):
                    for dtp in range(2):
                        ps2 = pso.tile([128, 2, D], F32, name="ps_o", tag="ps_o")
                        for i in range(2):
                            dt_ = 2 * dtp + i
                            for etp in range(2):
                                nc.tensor.matmul(
                                    ps2[:, i, 0:VG],
                                    lhsT=pt[hl][:, 2 * etp : 2 * etp + 2, dt_ * 128 : (dt_ + 1) * 128],
                                    rhs=v_p[:, 2 * etp : 2 * etp + 2, hl * VG : (hl + 1) * VG],
                                    start=(etp == 0),
                                    stop=(etp == 1),
                                    perf_mode=DR,
                                )
                        recip = rpool.tile([128, 2], F32, name="recip", tag="recip")
                        nc.vector.reciprocal(recip, ps2[:, :, NH : NH + 1])
                        for i in range(2):
                            dt_ = 2 * dtp + i
                            nc.vector.scalar_tensor_tensor(
                                out=z_p[:, dt_, hl * NH : (hl + 1) * NH],
                                in0=ps2[:, i, 0:NH],
                                scalar=recip[:, i : i + 1],
                                in1=xt["xqb"][:, dt_, hl * NH : (hl + 1) * NH],
                                op0=mybir.AluOpType.mult,
                                op1=mybir.AluOpType.add,
                            )
                        if pending_out:
                            pb, php, pz, groups = pending_out[0]
                            # During the final step, keep these drains off
                            # VectorE: the last pair's recip/STT chain is the
                            # critical path and must not queue behind them.
                            eng = (
                                "act"
                                if idx == NSTEP - 1
                                else ("act" if dr_tog[0] else "dve")
                            )
                            emit_outproj_group(pb, php, pz, groups.pop(0), eng=eng)
                            if not groups:
                                pending_out.pop(0)
                        yield
                pending_out.append((b, hp, z_p, list(range(PT))))

            # Software pipeline, woven at psum-group granularity: stage s1s2
            # of pair i runs alongside scores of pair i-1 and O of pair i-2,
            # so consecutive PE groups hit different PSUM pools and their
            # producers finished a full step earlier.
            WEAVE = ["a", "a", "a", "b", "c"] * 4
            for it_ in range(NSTEP + 2):
                if it_ == 0:
                    nc.vector.memset(warm, 0.0)
                    nc.scalar.activation(
                        out=bias_t,
                        in_=warm[:, 0:2].bitcast(F32),
                        func=mybir.ActivationFunctionType.Copy,
                        bias=EXP_BIAS,
                        scale=0.0,
                    )
                    # PE warmup on zeros: flip the HAM clock gate to 8/8 and
                    # ramp the p-state during the initial DMA window.
                    for _ in range(5):
                        ps_w = ps1.tile([128, D], F32, name="ps_p", tag="ps_p")
                        for _ in range(4):
                            nc.tensor.matmul(
                                ps_w, lhsT=warm[:, 0:128], rhs=warm, start=True, stop=True
                            )
                    # Startup: one DMA per tensor — each dma_start costs
                    # ~600ns of serial descriptor-gen on the sync engine, so
                    # fewer/bigger beats chunked (the first matmul needs the
                    # whole tensor anyway: contraction spans all of D).
                    load_w("wq")
                    load_x(0, "xq8")
                    load_w("wk")
                    load_x(0, "xk8")
                    load_w("wv")
                    load_x(0, "xv8")
                    load_x(0, "xqb")
                    # Batch 1 + wo enqueue on the idle gpsimd SWDGE queue so
                    # their descriptor-gen overlaps the sync engine's.
                    nc.gpsimd.dma_start(
                        out=w_sb["wo"],
                        in_=w_dram["wo"].rearrange("(t p) o -> p t o", p=128),
                    )
                    for nm in ("xq8", "xk8", "xv8", "xqb"):
                        nc.gpsimd.dma_start(out=x_sb[1][nm], in_=x_b[nm][1])
                gens = {}
                if it_ < NSTEP:
                    gens["a"] = gen_s1s2(it_)
                if 1 <= it_ <= NSTEP:
                    gens["b"] = gen_s3(it_ - 1)
                if 2 <= it_ <= NSTEP + 1:
                    gens["c"] = gen_s4(it_ - 2)
                for s in WEAVE:
                    if s in gens:
                        next(gens[s], None)
                for g in gens.values():
                    for _ in g:
                        pass

            for pb, php, pz, groups in pending_out:
                for g in groups:
                    emit_outproj_group(pb, php, pz, g, eng="act" if g % 2 == 0 else "dve")

    nc.compile()
    return nc


def _get_nc():
    global _NC_CACHE
    if _NC_CACHE is None:
        _NC_CACHE = build_nc()
    return _NC_CACHE


def _shard_inputs(inputs):
    xq = np.ascontiguousarray(np.asarray(inputs["X_Query"], dtype=np.float32))
    xk = np.ascontiguousarray(np.asarray(inputs["X_Key"], dtype=np.float32))
    xv = np.ascontiguousarray(np.asarray(inputs["X_Value"], dtype=np.float32))
    xq8 = xq.astype(NPF8)
    xqb = xq.astype(NPBF16)
    xk8 = xk.astype(NPF8)
    xv8 = xv.astype(NPF8)
    weights = {
        "wq8": np.ascontiguousarray(
            (np.asarray(inputs["W_q"], dtype=np.float32).T * WS).astype(NPF8)
        ),
        "wk8": np.ascontiguousarray(
            (np.asarray(inputs["W_k"], dtype=np.float32).T * WS).astype(NPF8)
        ),
        "wv8": np.ascontiguousarray(
            (np.asarray(inputs["W_v"], dtype=np.float32).T * WS).astype(NPF8)
        ),
        "wob": np.ascontiguousarray(
            np.asarray(inputs["W_o"], dtype=np.float32).T.astype(NPBF16)
        ),
    }
    in_maps = []
    for c in range(8):
        sl = slice(c * B_PER_CORE, (c + 1) * B_PER_CORE)
        in_maps.append(
            {
                "xq8": xq8[sl],
                "xqb": xqb[sl],
                "xk8": xk8[sl],
                "xv8": xv8[sl],
                **weights,
            }
        )
    return in_maps


def run_sharded(inputs, **kwargs):
    """Run on all 8 cores; returns (full_output, BassKernelResults)."""
    nc = _get_nc()
    in_maps = _shard_inputs(inputs)
    res = run_bass_kernel_spmd(nc, in_maps, core_ids=list(range(8)), **kwargs)
    full = np.concatenate(
        [np.asarray(r["out"]).astype(np.float32) for r in res.results], axis=0
    )
    return full, res


def kernel(**inputs):
    full, _ = run_sharded(inputs)
    return full


# revision 36
# speedup vs baseline: 1.1435x; 1.1435x over previous
"""Multi-head "channel attention" kernel for Trainium2 (8 NeuronCores).

Reference computation (B=16, D=512, N=2048, h=8 heads, Nh=256):
    q = Wq @ XQ ; k = Wk @ XK ; v = Wv @ XV          (per batch, (D,N))
    per head (N split into 8 chunks of 256):
      scores = q_h @ k_h^T / sqrt(Nh)                ((D,D), contract over Nh)
      p      = softmax(scores, axis=-1)
      o_h    = p @ v_h                               ((D,Nh), contract over D)
    attn = concat(o_h) ; out = Wo @ (XQ - attn)
Sharding: data-parallel over batch: 16 batches / 8 cores = 2 per core.

fp8 strategy (rel err ~7e-3 vs the 2e-2 gate, validated by emulation):
  * Every matmul except the final output projection runs in fp8e4m3 with
    MatmulPerfMode.DoubleRow: 256 contraction rows per instruction at
    1 cycle per output-free element = 2x the fp32r/bf16 rate.
  * The host ships XK/XV/XQ quantized to fp8 (plus a bf16 copy of XQ for
    the residual add), and Wq/Wk/Wv transposed and scaled by 16 in fp8
    (unscaled they'd sit in the subnormal range), Wo transposed in bf16.
  * The x16 weight scales cancel: exp(scale*x) uses scale 1/(256*16);
    V's guard columns hold -16 so PSUM accumulates -16*r, and
    Z = XQ + O * (-1/(16 r)) == XQ - attn.
  * exp also carries bias -ln(16), attenuating p~ = exp(s)/16 into
    e4m3's range (max |p| ~42 vs the 240 cap; scores are ~N(0,1) but the
    tail over 4M samples reaches ~6.5).
  * The attention term is only ~7% of Z's magnitude, so fp8 noise in the
    whole attention pipeline is diluted ~14x; the bf16 output projection
    sets the ~2e-3 floor.
  * Heads are processed in PAIRS (adjacent heads = adjacent n-columns),
    making most matmul frees and all drains 512 wide, halving fixed
    per-instruction costs. O-matmul free is 260 (odd frees like 258 run
    ~8x slow on the PE; 260 is even and full speed).
  * PSUM->SBUF drains are the co-bottleneck (DMA and GPSIMD have no PSUM
    port): split between ScalarE (exp, o, kt-half) and VectorE (qt, v,
    kt-half, STT, recip), each ~85us vs ~97us of PE work.
"""

import sys

if "/opt/trn_rl_repo" not in sys.path:
    sys.path.insert(0, "/opt/trn_rl_repo")

import numpy as np
import ml_dtypes

import concourse.bass as bass
import concourse.tile as tile
from concourse import bacc, mybir
from concourse.bass_utils import run_bass_kernel_spmd

B_PER_CORE = 2
D = 512
N = 2048
H = 8
NH = N // H  # 256
PT = D // 128  # 4 partition tiles over D
HP = H // 2  # 4 head pairs
NP = 2 * NH  # 512 columns per head pair

F32 = mybir.dt.float32
F8 = mybir.dt.float8e4
BF16 = mybir.dt.bfloat16
DR = mybir.MatmulPerfMode.DoubleRow

NPF8 = ml_dtypes.float8_e4m3
NPBF16 = ml_dtypes.bfloat16

WS = 16.0  # host-side weight scale for Wq/Wk/Wv in fp8
EXP_SCALE = 1.0 / (WS * WS * np.sqrt(NH))  # = 1/4096
EXP_BIAS = float(-np.log(16.0))  # p~ = exp(s)/16
VG = 260  # per-head V row: 256 data + 2 guard (-16) + 2 pad

_NC_CACHE = None


def build_nc():
    nc = bacc.Bacc("TRN2", target_bir_lowering=False, debug=False)

    xq8 = nc.dram_tensor("xq8", [B_PER_CORE, D, N], F8, kind="ExternalInput").ap()
    xqb = nc.dram_tensor("xqb", [B_PER_CORE, D, N], BF16, kind="ExternalInput").ap()
    xk8 = nc.dram_tensor("xk8", [B_PER_CORE, D, N], F8, kind="ExternalInput").ap()
    xv8 = nc.dram_tensor("xv8", [B_PER_CORE, D, N], F8, kind="ExternalInput").ap()
    wq8 = nc.dram_tensor("wq8", [D, D], F8, kind="ExternalInput").ap()
    wk8 = nc.dram_tensor("wk8", [D, D], F8, kind="ExternalInput").ap()
    wv8 = nc.dram_tensor("wv8", [D, D], F8, kind="ExternalInput").ap()
    wob = nc.dram_tensor("wob", [D, D], BF16, kind="ExternalInput").ap()
    out = nc.dram_tensor("out", [B_PER_CORE, D, N], BF16, kind="ExternalOutput").ap()

    with tile.TileContext(nc) as tc:
        with (
            tc.tile_pool(name="wpool", bufs=1) as wpool,
            tc.tile_pool(name="xpool", bufs=1) as xpool,
            tc.tile_pool(name="qkpool", bufs=6) as qkpool,
            tc.tile_pool(name="vpool", bufs=4) as vpool,
            tc.tile_pool(name="ptpool", bufs=6) as ptpool,
            tc.tile_pool(name="zpool", bufs=3) as zpool,
            tc.tile_pool(name="opool", bufs=6) as opool,
            tc.tile_pool(name="rpool", bufs=8) as rpool,
            tc.tile_pool(name="ps1", bufs=4, space="PSUM") as ps1,
            tc.tile_pool(name="psS", bufs=1, space="PSUM") as psS,
            tc.tile_pool(name="pso", bufs=1, space="PSUM") as pso,
        ):
            # Resident weights: [p, it, o] = W.T[it*128+p, o] (fp8 x16),
            # loaded in per-it chunks to shorten the first matmul's dep.
            w_sb = {
                "wq": wpool.tile([128, PT, D], F8, name="w_wq", tag="w_wq"),
                "wk": wpool.tile([128, PT, D], F8, name="w_wk", tag="w_wk"),
                "wv": wpool.tile([128, PT, D], F8, name="w_wv", tag="w_wv"),
                "wo": wpool.tile([128, PT, D], BF16, name="w_wo", tag="w_wo"),
            }
            w_dram = {"wq": wq8, "wk": wk8, "wv": wv8, "wo": wob}

            def load_w(name):
                src = w_dram[name].rearrange("(t p) o -> p t o", p=128)
                nc.sync.dma_start(out=w_sb[name], in_=src)

            x_b = {
                "xq8": [xq8[b].rearrange("(t p) n -> p t n", p=128) for b in range(B_PER_CORE)],
                "xqb": [xqb[b].rearrange("(t p) n -> p t n", p=128) for b in range(B_PER_CORE)],
                "xk8": [xk8[b].rearrange("(t p) n -> p t n", p=128) for b in range(B_PER_CORE)],
                "xv8": [xv8[b].rearrange("(t p) n -> p t n", p=128) for b in range(B_PER_CORE)],
            }
            x_dt = {"xq8": F8, "xqb": BF16, "xk8": F8, "xv8": F8}

            # Whole-batch input tiles, DMA'd in per-it chunks (256-512 KiB,
            # 2 KiB bursts) up front; per-pair slices view into these.
            x_sb = [
                {
                    nm: xpool.tile(
                        [128, PT, N], x_dt[nm], name=f"{nm}_b{b}", tag=f"{nm}_b{b}"
                    )
                    for nm in ("xq8", "xqb", "xk8", "xv8")
                }
                for b in range(B_PER_CORE)
            ]

            def load_x(b, nm):
                nc.sync.dma_start(out=x_sb[b][nm], in_=x_b[nm][b])

            steps = [(b, hp) for b in range(B_PER_CORE) for hp in range(HP)]
            NSTEP = len(steps)
            # per-step live tiles for the software pipeline
            st_qt = {}
            st_kt = {}
            st_v = {}
            st_pt = {}
            # (b, hp, z_pair) whose output projection hasn't been emitted yet
            pending_out = []

            # Output DMAs are merged (all four dt groups per dma_start) to
            # cut the ~600ns/DMA sync-engine descriptor-gen cost.
            o_pair = {}

            def emit_outproj_group(b, hp, z_p, dt_, eng="act"):
                """One [128, 512] output-projection group (bf16)."""
                out_b = out[b].rearrange("(t p) n -> p t n", p=128)
                ns_ = slice(hp * NP, (hp + 1) * NP)
                ps = ps1.tile([128, D], F32, name="ps_p", tag="ps_p")
                for it in range(PT):
                    nc.tensor.matmul(
                        ps,
                        lhsT=w_sb["wo"][:, it, dt_ * 128 : (dt_ + 1) * 128],
                        rhs=z_p[:, it, :],
                        start=(it == 0),
                        stop=(it == PT - 1),
                    )
                if dt_ == 0:
                    o_pair[(b, hp)] = opool.tile([128, PT, D], BF16, name="o_sb", tag="o_sb")
                o_sb = o_pair[(b, hp)]
                if eng == "act":
                    nc.scalar.copy(out=o_sb[:, dt_, :], in_=ps)
                else:
                    nc.vector.tensor_copy(out=o_sb[:, dt_, :], in_=ps)
                if dt_ == PT - 1:
                    nc.sync.dma_start(
                        out=out_b[:, :, ns_], in_=o_pair.pop((b, hp))
                    )

            warm = wpool.tile([128, D], BF16, name="warm", tag="warm")
            bias_t = wpool.tile([128, 1], F32, name="bias_t", tag="bias_t")

            def xs(idx):
                b, hp = steps[idx]
                ns = slice(hp * NP, (hp + 1) * NP)
                return {nm: t[:, :, ns] for nm, t in x_sb[b].items()}

            # ps1 drains alternate between ScalarE and VectorE per group.
            dr_tog = [0]

            def drain(dst, src):
                dr_tog[0] ^= 1
                if dr_tog[0]:
                    nc.scalar.copy(out=dst, in_=src)
                else:
                    nc.vector.tensor_copy(out=dst, in_=src)

            def gen_s1s2(idx):
                """12 psum-groups: QT/KT (8) then V (4), all fp8 DR."""
                xt = xs(idx)
                st_qt[idx] = []
                st_kt[idx] = []
                for hl in range(2):
                    qt_h = qkpool.tile([128, 2, D], F8, name="qt_h", tag="qt_h")
                    kt_h = qkpool.tile([128, 2, D], F8, name="kt_h", tag="kt_h")
                    for dst, src, w in ((qt_h, xt["xq8"], "wq"), (kt_h, xt["xk8"], "wk")):
                        for jt in range(2):
                            nb = hl * NH + jt * 128
                            ps = ps1.tile([128, D], F32, name="ps_p", tag="ps_p")
                            for itp in range(2):
                                nc.tensor.matmul(
                                    ps,
                                    lhsT=src[:, 2 * itp : 2 * itp + 2, nb : nb + 128],
                                    rhs=w_sb[w][:, 2 * itp : 2 * itp + 2, :],
                                    start=(itp == 0),
                                    stop=(itp == 1),
                                    perf_mode=DR,
                                )
                            drain(dst[:, jt, :], ps)
                            yield
                    st_qt[idx].append(qt_h)
                    st_kt[idx].append(kt_h)
                v_p = vpool.tile([128, PT, 2 * VG], F8, name="v_p", tag="v_p")
                st_v[idx] = v_p
                guard = v_p.rearrange("p a (g c) -> p (a g) c", c=VG)[:, :, NH:VG]
                gw = VG - NH
                nc.scalar.activation(
                    out=guard,
                    in_=w_sb["wv"][:, 0, 0 : 8 * gw].rearrange("p (a c) -> p a c", c=gw),
                    func=mybir.ActivationFunctionType.Copy,
                    bias=-16.0,
                    scale=0.0,
                )
                for et in range(PT):
                    ps = ps1.tile([128, D], F32, name="ps_p", tag="ps_p")
                    for itp in range(2):
                        nc.tensor.matmul(
                            ps,
                            lhsT=w_sb["wv"][:, 2 * itp : 2 * itp + 2, et * 128 : (et + 1) * 128],
                            rhs=xt["xv8"][:, 2 * itp : 2 * itp + 2, :],
                            start=(itp == 0),
                            stop=(itp == 1),
                            perf_mode=DR,
                        )
                    dst = v_p[:, et, :].rearrange("p (g c) -> p g c", c=VG)[:, :, 0:NH]
                    drain(dst, ps.rearrange("p (g c) -> p g c", c=NH))
                    yield

            def gen_s3(idx):
                """4 psum-groups: scores^T into 2-bank tiles, wide exp."""
                qt, kt = st_qt.pop(idx), st_kt.pop(idx)
                st_pt[idx] = []
                for hl in range(2):
                    pt_h = ptpool.tile([128, PT, D], F8, name="pt_h", tag="pt_h")
                    for ep in range(2):
                        ps = psS.tile([128, 2, D], F32, name="ps_s", tag="ps_s")
                        for i in range(2):
                            et = 2 * ep + i
                            nc.tensor.matmul(
                                ps[:, i, :],
                                lhsT=kt[hl][:, 0:2, et * 128 : (et + 1) * 128],
                                rhs=qt[hl][:, 0:2, :],
                                start=True,
                                stop=True,
                                perf_mode=DR,
                            )
                        nc.scalar.activation(
                            out=pt_h[:, 2 * ep : 2 * ep + 2, :],
                            in_=ps,
                            func=mybir.ActivationFunctionType.Exp,
                            scale=float(EXP_SCALE),
                            bias=bias_t,
                        )
                        yield
                    st_pt[idx].append(pt_h)

            def gen_s4(idx):
                """4 psum-groups: O + recip + STT; consumes one pending
                output-projection group after each."""
                b, hp = steps[idx]
                xt = xs(idx)
                pt = st_pt.pop(idx)
                v_p = st_v.pop(idx)
                z_p = zpool.tile([128, PT, NP], BF16, name="z_p", tag="z_p")
                for hl in range(2):
                    for dtp in range(2):
                        ps2 = pso.tile([128, 2, D], F32, name="ps_o", tag="ps_o")
                        for i in range(2):
                            dt_ = 2 * dtp + i
                            for etp in range(2):
                                nc.tensor.matmul(
                                    ps2[:, i, 0:VG],
                                    lhsT=pt[hl][:, 2 * etp : 2 * etp + 2, dt_ * 128 : (dt_ + 1) * 128],
                                    rhs=v_p[:, 2 * etp : 2 * etp + 2, hl * VG : (hl + 1) * VG],
                                    start=(etp == 0),
                                    stop=(etp == 1),
                                    perf_mode=DR,
                                )
                        recip = rpool.tile([128, 2], F32, name="recip", tag="recip")
                        nc.vector.reciprocal(recip, ps2[:, :, NH : NH + 1])
                        for i in range(2):
                            dt_ = 2 * dtp + i
                            nc.vector.scalar_tensor_tensor(
                                out=z_p[:, dt_, hl * NH : (hl + 1) * NH],
                                in0=ps2[:, i, 0:NH],
                                scalar=recip[:, i : i + 1],
                                in1=xt["xqb"][:, dt_, hl * NH : (hl + 1) * NH],
                                op0=mybir.AluOpType.mult,
                                op1=mybir.AluOpType.add,
                            )
                        if pending_out:
                            pb, php, pz, groups = pending_out[0]
                            # During the final step, keep these drains off
                            # VectorE: the last pair's recip/STT chain is the
                            # critical path and must not queue behind them.
                            eng = (
                                "act"
                                if idx == NSTEP - 1
                                else ("act" if dr_tog[0] else "dve")
                            )
                            emit_outproj_group(pb, php, pz, groups.pop(0), eng=eng)
                            if not groups:
                                pending_out.pop(0)
                        yield
                pending_out.append((b, hp, z_p, list(range(PT))))

            # Software pipeline, woven at psum-group granularity: stage s1s2
            # of pair i runs alongside scores of pair i-1 and O of pair i-2,
            # so consecutive PE groups hit different PSUM pools and their
            # producers finished a full step earlier.
            WEAVE = ["a", "a", "a", "b", "c"] * 4
            for it_ in range(NSTEP + 2):
                if it_ == 0:
                    nc.vector.memset(warm, 0.0)
                    nc.scalar.activation(
                        out=bias_t,
                        in_=warm[:, 0:2].bitcast(F32),
                        func=mybir.ActivationFunctionType.Copy,
                        bias=EXP_BIAS,
                        scale=0.0,
                    )
                    # PE warmup on zeros: flip the HAM clock gate to 8/8 and
                    # ramp the p-state during the initial DMA window.
                    for _ in range(5):
                        ps_w = ps1.tile([128, D], F32, name="ps_p", tag="ps_p")
                        for _ in range(4):
                            nc.tensor.matmul(
                                ps_w, lhsT=warm[:, 0:128], rhs=warm, start=True, stop=True
                            )
                    # Startup: one DMA per tensor — each dma_start costs
                    # ~600ns of serial descriptor-gen on the sync engine, so
                    # fewer/bigger beats chunked (the first matmul needs the
                    # whole tensor anyway: contraction spans all of D).
                    load_w("wq")
                    load_x(0, "xq8")
                    load_w("wk")
                    load_x(0, "xk8")
                    load_w("wv")
                    load_x(0, "xv8")
                    load_x(0, "xqb")
                    load_w("wo")
                    load_x(1, "xq8")
                    load_x(1, "xk8")
                    load_x(1, "xv8")
                    load_x(1, "xqb")
                gens = {}
                if it_ < NSTEP:
                    gens["a"] = gen_s1s2(it_)
                if 1 <= it_ <= NSTEP:
                    gens["b"] = gen_s3(it_ - 1)
                if 2 <= it_ <= NSTEP + 1:
                    gens["c"] = gen_s4(it_ - 2)
                for s in WEAVE:
                    if s in gens:
                        next(gens[s], None)
                for g in gens.values():
                    for _ in g:
                        pass

            for pb, php, pz, groups in pending_out:
                for g in groups:
                    emit_outproj_group(pb, php, pz, g, eng="act" if g % 2 == 0 else "dve")

    nc.compile()
    return nc


def _get_nc():
    global _NC_CACHE
    if _NC_CACHE is None:
        _NC_CACHE = build_nc()
    return _NC_CACHE


def _shard_inputs(inputs):
    xq = np.ascontiguousarray(np.asarray(inputs["X_Query"], dtype=np.float32))
    xk = np.ascontiguousarray(np.asarray(inputs["X_Key"], dtype=np.float32))
    xv = np.ascontiguousarray(np.asarray(inputs["X_Value"], dtype=np.float32))
    xq8 = xq.astype(NPF8)
    xqb = xq.astype(NPBF16)
    xk8 = xk.astype(NPF8)
    xv8 = xv.astype(NPF8)
    weights = {
        "wq8": np.ascontiguousarray(
            (np.asarray(inputs["W_q"], dtype=np.float32).T * WS).astype(NPF8)
        ),
        "wk8": np.ascontiguousarray(
            (np.asarray(inputs["W_k"], dtype=np.float32).T * WS).astype(NPF8)
        ),
        "wv8": np.ascontiguousarray(
            (np.asarray(inputs["W_v"], dtype=np.float32).T * WS).astype(NPF8)
        ),
        "wob": np.ascontiguousarray(
            np.asarray(inputs["W_o"], dtype=np.float32).T.astype(NPBF16)
        ),
    }
    in_maps = []
    for c in range(8):
        sl = slice(c * B_PER_CORE, (c + 1) * B_PER_CORE)
        in_maps.append(
            {
                "xq8": xq8[sl],
                "xqb": xqb[sl],
                "xk8": xk8[sl],
                "xv8": xv8[sl],
                **weights,
            }
        )
    return in_maps


def run_sharded(inputs, **kwargs):
    """Run on all 8 cores; returns (full_output, BassKernelResults)."""
    nc = _get_nc()
    in_maps = _shard_inputs(inputs)
    res = run_bass_kernel_spmd(nc, in_maps, core_ids=list(range(8)), **kwargs)
    full = np.concatenate(
        [np.asarray(r["out"]).astype(np.float32) for r in res.results], axis=0
    )
    return full, res


def kernel(**inputs):
    full, _ = run_sharded(inputs)
    return full


# revision 38
# speedup vs baseline: 1.1529x; 1.0082x over previous
"""Multi-head "channel attention" kernel for Trainium2 (8 NeuronCores).

Reference computation (B=16, D=512, N=2048, h=8 heads, Nh=256):
    q = Wq @ XQ ; k = Wk @ XK ; v = Wv @ XV          (per batch, (D,N))
    per head (N split into 8 chunks of 256):
      scores = q_h @ k_h^T / sqrt(Nh)                ((D,D), contract over Nh)
      p      = softmax(scores, axis=-1)
      o_h    = p @ v_h                               ((D,Nh), contract over D)
    attn = concat(o_h) ; out = Wo @ (XQ - attn)
Sharding: data-parallel over batch: 16 batches / 8 cores = 2 per core.

fp8 strategy (rel err ~7e-3 vs the 2e-2 gate, validated by emulation):
  * Every matmul except the final output projection runs in fp8e4m3 with
    MatmulPerfMode.DoubleRow: 256 contraction rows per instruction at
    1 cycle per output-free element = 2x the fp32r/bf16 rate.
  * The host ships XK/XV/XQ quantized to fp8 (plus a bf16 copy of XQ for
    the residual add), and Wq/Wk/Wv transposed and scaled by 16 in fp8
    (unscaled they'd sit in the subnormal range), Wo transposed in bf16.
  * The x16 weight scales cancel: exp(scale*x) uses scale 1/(256*16);
    V's guard columns hold -16 so PSUM accumulates -16*r, and
    Z = XQ + O * (-1/(16 r)) == XQ - attn.
  * exp also carries bias -ln(16), attenuating p~ = exp(s)/16 into
    e4m3's range (max |p| ~42 vs the 240 cap; scores are ~N(0,1) but the
    tail over 4M samples reaches ~6.5).
  * The attention term is only ~7% of Z's magnitude, so fp8 noise in the
    whole attention pipeline is diluted ~14x; the bf16 output projection
    sets the ~2e-3 floor.
  * Heads are processed in PAIRS (adjacent heads = adjacent n-columns),
    making most matmul frees and all drains 512 wide, halving fixed
    per-instruction costs. O-matmul free is 260 (odd frees like 258 run
    ~8x slow on the PE; 260 is even and full speed).
  * PSUM->SBUF drains are the co-bottleneck (DMA and GPSIMD have no PSUM
    port): split between ScalarE (exp, o, kt-half) and VectorE (qt, v,
    kt-half, STT, recip), each ~85us vs ~97us of PE work.
"""

import sys

if "/opt/trn_rl_repo" not in sys.path:
    sys.path.insert(0, "/opt/trn_rl_repo")

import numpy as np
import ml_dtypes

import concourse.bass as bass
import concourse.tile as tile
from concourse import bacc, mybir
from concourse.bass_utils import run_bass_kernel_spmd

B_PER_CORE = 2
D = 512
N = 2048
H = 8
NH = N // H  # 256
PT = D // 128  # 4 partition tiles over D
HP = H // 2  # 4 head pairs
NP = 2 * NH  # 512 columns per head pair

F32 = mybir.dt.float32
F8 = mybir.dt.float8e4
BF16 = mybir.dt.bfloat16
DR = mybir.MatmulPerfMode.DoubleRow

NPF8 = ml_dtypes.float8_e4m3
NPBF16 = ml_dtypes.bfloat16

WS = 16.0  # host-side weight scale for Wq/Wk/Wv in fp8
EXP_SCALE = 1.0 / (WS * WS * np.sqrt(NH))  # = 1/4096
EXP_BIAS = float(-np.log(16.0))  # p~ = exp(s)/16
VG = 260  # per-head V row: 256 data + 2 guard (-16) + 2 pad

_NC_CACHE = None


def build_nc():
    nc = bacc.Bacc("TRN2", target_bir_lowering=False, debug=False)

    xq8 = nc.dram_tensor("xq8", [B_PER_CORE, D, N], F8, kind="ExternalInput").ap()
    xqb = nc.dram_tensor("xqb", [B_PER_CORE, D, N], BF16, kind="ExternalInput").ap()
    xk8 = nc.dram_tensor("xk8", [B_PER_CORE, D, N], F8, kind="ExternalInput").ap()
    xv8 = nc.dram_tensor("xv8", [B_PER_CORE, D, N], F8, kind="ExternalInput").ap()
    wq8 = nc.dram_tensor("wq8", [D, D], F8, kind="ExternalInput").ap()
    wk8 = nc.dram_tensor("wk8", [D, D], F8, kind="ExternalInput").ap()
    wv8 = nc.dram_tensor("wv8", [D, D], F8, kind="ExternalInput").ap()
    wob = nc.dram_tensor("wob", [D, D], BF16, kind="ExternalInput").ap()
    out = nc.dram_tensor("out", [B_PER_CORE, D, N], BF16, kind="ExternalOutput").ap()

    with tile.TileContext(nc) as tc:
        with (
            tc.tile_pool(name="wpool", bufs=1) as wpool,
            tc.tile_pool(name="xpool", bufs=1) as xpool,
            tc.tile_pool(name="qkpool", bufs=6) as qkpool,
            tc.tile_pool(name="vpool", bufs=4) as vpool,
            tc.tile_pool(name="ptpool", bufs=6) as ptpool,
            tc.tile_pool(name="zpool", bufs=3) as zpool,
            tc.tile_pool(name="opool", bufs=6) as opool,
            tc.tile_pool(name="rpool", bufs=8) as rpool,
            tc.tile_pool(name="ps1", bufs=4, space="PSUM") as ps1,
            tc.tile_pool(name="psS", bufs=1, space="PSUM") as psS,
            tc.tile_pool(name="pso", bufs=1, space="PSUM") as pso,
        ):
            # Resident weights: [p, it, o] = W.T[it*128+p, o] (fp8 x16),
            # loaded in per-it chunks to shorten the first matmul's dep.
            w_sb = {
                "wq": wpool.tile([128, PT, D], F8, name="w_wq", tag="w_wq"),
                "wk": wpool.tile([128, PT, D], F8, name="w_wk", tag="w_wk"),
                "wv": wpool.tile([128, PT, D], F8, name="w_wv", tag="w_wv"),
                "wo": wpool.tile([128, PT, D], BF16, name="w_wo", tag="w_wo"),
            }
            w_dram = {"wq": wq8, "wk": wk8, "wv": wv8, "wo": wob}

            def load_w(name):
                src = w_dram[name].rearrange("(t p) o -> p t o", p=128)
                nc.sync.dma_start(out=w_sb[name], in_=src)

            x_b = {
                "xq8": [xq8[b].rearrange("(t p) n -> p t n", p=128) for b in range(B_PER_CORE)],
                "xqb": [xqb[b].rearrange("(t p) n -> p t n", p=128) for b in range(B_PER_CORE)],
                "xk8": [xk8[b].rearrange("(t p) n -> p t n", p=128) for b in range(B_PER_CORE)],
                "xv8": [xv8[b].rearrange("(t p) n -> p t n", p=128) for b in range(B_PER_CORE)],
            }
            x_dt = {"xq8": F8, "xqb": BF16, "xk8": F8, "xv8": F8}

            # Whole-batch input tiles, DMA'd in per-it chunks (256-512 KiB,
            # 2 KiB bursts) up front; per-pair slices view into these.
            x_sb = [
                {
                    nm: xpool.tile(
                        [128, PT, N], x_dt[nm], name=f"{nm}_b{b}", tag=f"{nm}_b{b}"
                    )
                    for nm in ("xq8", "xqb", "xk8", "xv8")
                }
                for b in range(B_PER_CORE)
            ]

            def load_x(b, nm):
                nc.sync.dma_start(out=x_sb[b][nm], in_=x_b[nm][b])

            steps = [(b, hp) for b in range(B_PER_CORE) for hp in range(HP)]
            NSTEP = len(steps)
            # per-step live tiles for the software pipeline
            st_qt = {}
            st_kt = {}
            st_v = {}
            st_pt = {}
            # (b, hp, z_pair) whose output projection hasn't been emitted yet
            pending_out = []

            # Output DMAs are paired (two dt groups per dma_start) to halve
            # the ~600ns/DMA sync-engine descriptor-gen cost.
            o_pair = {}

            def emit_outproj_group(b, hp, z_p, dt_, eng="act"):
                """One [128, 512] output-projection group (bf16)."""
                out_b = out[b].rearrange("(t p) n -> p t n", p=128)
                ns_ = slice(hp * NP, (hp + 1) * NP)
                ps = ps1.tile([128, D], F32, name="ps_p", tag="ps_p")
                for it in range(PT):
                    nc.tensor.matmul(
                        ps,
                        lhsT=w_sb["wo"][:, it, dt_ * 128 : (dt_ + 1) * 128],
                        rhs=z_p[:, it, :],
                        start=(it == 0),
                        stop=(it == PT - 1),
                    )
                if dt_ % 2 == 0:
                    o_pair[(b, hp)] = opool.tile([128, 2, D], BF16, name="o_sb", tag="o_sb")
                o_sb = o_pair[(b, hp)]
                if eng == "act":
                    nc.scalar.copy(out=o_sb[:, dt_ % 2, :], in_=ps)
                else:
                    nc.vector.tensor_copy(out=o_sb[:, dt_ % 2, :], in_=ps)
                if dt_ % 2 == 1:
                    nc.sync.dma_start(
                        out=out_b[:, dt_ - 1 : dt_ + 1, ns_],
                        in_=o_pair.pop((b, hp)),
                    )

            warm = wpool.tile([128, D], BF16, name="warm", tag="warm")
            bias_t = wpool.tile([128, 1], F32, name="bias_t", tag="bias_t")

            def xs(idx):
                b, hp = steps[idx]
                ns = slice(hp * NP, (hp + 1) * NP)
                return {nm: t[:, :, ns] for nm, t in x_sb[b].items()}

            # ps1 drains alternate between ScalarE and VectorE per group.
            dr_tog = [0]

            def drain(dst, src):
                dr_tog[0] ^= 1
                if dr_tog[0]:
                    nc.scalar.copy(out=dst, in_=src)
                else:
                    nc.vector.tensor_copy(out=dst, in_=src)

            def gen_s1s2(idx):
                """12 psum-groups: QT/KT (8) then V (4), all fp8 DR."""
                xt = xs(idx)
                st_qt[idx] = []
                st_kt[idx] = []
                for hl in range(2):
                    qt_h = qkpool.tile([128, 2, D], F8, name="qt_h", tag="qt_h")
                    kt_h = qkpool.tile([128, 2, D], F8, name="kt_h", tag="kt_h")
                    for dst, src, w in ((qt_h, xt["xq8"], "wq"), (kt_h, xt["xk8"], "wk")):
                        for jt in range(2):
                            nb = hl * NH + jt * 128
                            ps = ps1.tile([128, D], F32, name="ps_p", tag="ps_p")
                            for itp in range(2):
                                nc.tensor.matmul(
                                    ps,
                                    lhsT=src[:, 2 * itp : 2 * itp + 2, nb : nb + 128],
                                    rhs=w_sb[w][:, 2 * itp : 2 * itp + 2, :],
                                    start=(itp == 0),
                                    stop=(itp == 1),
                                    perf_mode=DR,
                                )
                            drain(dst[:, jt, :], ps)
                            yield
                    st_qt[idx].append(qt_h)
                    st_kt[idx].append(kt_h)
                v_p = vpool.tile([128, PT, 2 * VG], F8, name="v_p", tag="v_p")
                st_v[idx] = v_p
                guard = v_p.rearrange("p a (g c) -> p (a g) c", c=VG)[:, :, NH:VG]
                gw = VG - NH
                nc.scalar.activation(
                    out=guard,
                    in_=w_sb["wv"][:, 0, 0 : 8 * gw].rearrange("p (a c) -> p a c", c=gw),
                    func=mybir.ActivationFunctionType.Copy,
                    bias=-16.0,
                    scale=0.0,
                )
                for et in range(PT):
                    ps = ps1.tile([128, D], F32, name="ps_p", tag="ps_p")
                    for itp in range(2):
                        nc.tensor.matmul(
                            ps,
                            lhsT=w_sb["wv"][:, 2 * itp : 2 * itp + 2, et * 128 : (et + 1) * 128],
                            rhs=xt["xv8"][:, 2 * itp : 2 * itp + 2, :],
                            start=(itp == 0),
                            stop=(itp == 1),
                            perf_mode=DR,
                        )
                    dst = v_p[:, et, :].rearrange("p (g c) -> p g c", c=VG)[:, :, 0:NH]
                    drain(dst, ps.rearrange("p (g c) -> p g c", c=NH))
                    yield

            def gen_s3(idx):
                """4 psum-groups: scores^T into 2-bank tiles, wide exp."""
                qt, kt = st_qt.pop(idx), st_kt.pop(idx)
                st_pt[idx] = []
                for hl in range(2):
                    pt_h = ptpool.tile([128, PT, D], F8, name="pt_h", tag="pt_h")
                    for ep in range(2):
                        ps = psS.tile([128, 2, D], F32, name="ps_s", tag="ps_s")
                        for i in range(2):
                            et = 2 * ep + i
                            nc.tensor.matmul(
                                ps[:, i, :],
                                lhsT=kt[hl][:, 0:2, et * 128 : (et + 1) * 128],
                                rhs=qt[hl][:, 0:2, :],
                                start=True,
                                stop=True,
                                perf_mode=DR,
                            )
                        nc.scalar.activation(
                            out=pt_h[:, 2 * ep : 2 * ep + 2, :],
                            in_=ps,
                            func=mybir.ActivationFunctionType.Exp,
                            scale=float(EXP_SCALE),
                            bias=bias_t,
                        )
                        yield
                    st_pt[idx].append(pt_h)

            def gen_s4(idx):
                """4 psum-groups: O + recip + STT; consumes one pending
                output-projection group after each."""
                b, hp = steps[idx]
                xt = xs(idx)
                pt = st_pt.pop(idx)
                v_p = st_v.pop(idx)
                z_p = zpool.tile([128, PT, NP], BF16, name="z_p", tag="z_p")
                for hl in range(2):
                    for dtp in range(2):
                        ps2 = pso.tile([128, 2, D], F32, name="ps_o", tag="ps_o")
                        for i in range(2):
                            dt_ = 2 * dtp + i
                            for etp in range(2):
                                nc.tensor.matmul(
                                    ps2[:, i, 0:VG],
                                    lhsT=pt[hl][:, 2 * etp : 2 * etp + 2, dt_ * 128 : (dt_ + 1) * 128],
                                    rhs=v_p[:, 2 * etp : 2 * etp + 2, hl * VG : (hl + 1) * VG],
                                    start=(etp == 0),
                                    stop=(etp == 1),
                                    perf_mode=DR,
                                )
                        recip = rpool.tile([128, 2], F32, name="recip", tag="recip")
                        nc.vector.reciprocal(recip, ps2[:, :, NH : NH + 1])
                        for i in range(2):
                            dt_ = 2 * dtp + i
                            nc.vector.scalar_tensor_tensor(
                                out=z_p[:, dt_, hl * NH : (hl + 1) * NH],
                                in0=ps2[:, i, 0:NH],
                                scalar=recip[:, i : i + 1],
                                in1=xt["xqb"][:, dt_, hl * NH : (hl + 1) * NH],
                                op0=mybir.AluOpType.mult,
                                op1=mybir.AluOpType.add,
                            )
                        if pending_out:
                            pb, php, pz, groups = pending_out[0]
                            # During the final step, keep these drains off
                            # VectorE: the last pair's recip/STT chain is the
                            # critical path and must not queue behind them.
                            eng = (
                                "act"
                                if idx == NSTEP - 1
                                else ("act" if dr_tog[0] else "dve")
                            )
                            emit_outproj_group(pb, php, pz, groups.pop(0), eng=eng)
                            if not groups:
                                pending_out.pop(0)
                        yield
                pending_out.append((b, hp, z_p, list(range(PT))))

            # Software pipeline, woven at psum-group granularity: stage s1s2
            # of pair i runs alongside scores of pair i-1 and O of pair i-2,
            # so consecutive PE groups hit different PSUM pools and their
            # producers finished a full step earlier.
            WEAVE = ["a", "a", "a", "b", "c"] * 4
            for it_ in range(NSTEP + 2):
                if it_ == 0:
                    nc.vector.memset(warm, 0.0)
                    nc.scalar.activation(
                        out=bias_t,
                        in_=warm[:, 0:2].bitcast(F32),
                        func=mybir.ActivationFunctionType.Copy,
                        bias=EXP_BIAS,
                        scale=0.0,
                    )
                    # PE warmup on zeros: flip the HAM clock gate to 8/8 and
                    # ramp the p-state during the initial DMA window.
                    for _ in range(5):
                        ps_w = ps1.tile([128, D], F32, name="ps_p", tag="ps_p")
                        for _ in range(4):
                            nc.tensor.matmul(
                                ps_w, lhsT=warm[:, 0:128], rhs=warm, start=True, stop=True
                            )
                    # Startup: one DMA per tensor — each dma_start costs
                    # ~600ns of serial descriptor-gen on the sync engine, so
                    # fewer/bigger beats chunked (the first matmul needs the
                    # whole tensor anyway: contraction spans all of D).
                    load_w("wq")
                    load_x(0, "xq8")
                    load_w("wk")
                    load_x(0, "xk8")
                    load_w("wv")
                    load_x(0, "xv8")
                    load_x(0, "xqb")
                    load_w("wo")
                    load_x(1, "xq8")
                    load_x(1, "xk8")
                    load_x(1, "xv8")
                    load_x(1, "xqb")
                gens = {}
                if it_ < NSTEP:
                    gens["a"] = gen_s1s2(it_)
                if 1 <= it_ <= NSTEP:
                    gens["b"] = gen_s3(it_ - 1)
                if 2 <= it_ <= NSTEP + 1:
                    gens["c"] = gen_s4(it_ - 2)
                for s in WEAVE:
                    if s in gens:
                        next(gens[s], None)
                for g in gens.values():
                    for _ in g:
                        pass

            for pb, php, pz, groups in pending_out:
                for g in groups:
                    emit_outproj_group(pb, php, pz, g, eng="act" if g % 2 == 0 else "dve")

    nc.compile()
    return nc


def _get_nc():
    global _NC_CACHE
    if _NC_CACHE is None:
        _NC_CACHE = build_nc()
    return _NC_CACHE


def _shard_inputs(inputs):
    xq = np.ascontiguousarray(np.asarray(inputs["X_Query"], dtype=np.float32))
    xk = np.ascontiguousarray(np.asarray(inputs["X_Key"], dtype=np.float32))
    xv = np.ascontiguousarray(np.asarray(inputs["X_Value"], dtype=np.float32))
    xq8 = xq.astype(NPF8)
    xqb = xq.astype(NPBF16)
    xk8 = xk.astype(NPF8)
    xv8 = xv.astype(NPF8)
    weights = {
        "wq8": np.ascontiguousarray(
            (np.asarray(inputs["W_q"], dtype=np.float32).T * WS).astype(NPF8)
        ),
        "wk8": np.ascontiguousarray(
            (np.asarray(inputs["W_k"], dtype=np.float32).T * WS).astype(NPF8)
        ),
        "wv8": np.ascontiguousarray(
            (np.asarray(inputs["W_v"], dtype=np.float32).T * WS).astype(NPF8)
        ),
        "wob": np.ascontiguousarray(
            np.asarray(inputs["W_o"], dtype=np.float32).T.astype(NPBF16)
        ),
    }
    in_maps = []
    for c in range(8):
        sl = slice(c * B_PER_CORE, (c + 1) * B_PER_CORE)
        in_maps.append(
            {
                "xq8": xq8[sl],
                "xqb": xqb[sl],
                "xk8": xk8[sl],
                "xv8": xv8[sl],
                **weights,
            }
        )
    return in_maps


def run_sharded(inputs, **kwargs):
    """Run on all 8 cores; returns (full_output, BassKernelResults)."""
    nc = _get_nc()
    in_maps = _shard_inputs(inputs)
    res = run_bass_kernel_spmd(nc, in_maps, core_ids=list(range(8)), **kwargs)
    full = np.concatenate(
        [np.asarray(r["out"]).astype(np.float32) for r in res.results], axis=0
    )
    return full, res


def kernel(**inputs):
    full, _ = run_sharded(inputs)
    return full


# revision 43
# speedup vs baseline: 1.2198x; 1.0580x over previous
"""Multi-head "channel attention" kernel for Trainium2 (8 NeuronCores).

Reference computation (B=16, D=512, N=2048, h=8 heads, Nh=256):
    q = Wq @ XQ ; k = Wk @ XK ; v = Wv @ XV          (per batch, (D,N))
    per head (N split into 8 chunks of 256):
      scores = q_h @ k_h^T / sqrt(Nh)                ((D,D), contract over Nh)
      p      = softmax(scores, axis=-1)
      o_h    = p @ v_h                               ((D,Nh), contract over D)
    attn = concat(o_h) ; out = Wo @ (XQ - attn)
Sharding: data-parallel over batch: 16 batches / 8 cores = 2 per core.

fp8 strategy (rel err ~7e-3 vs the 2e-2 gate, validated by emulation):
  * Every matmul except the final output projection runs in fp8e4m3 with
    MatmulPerfMode.DoubleRow: 256 contraction rows per instruction at
    1 cycle per output-free element = 2x the fp32r/bf16 rate.
  * The host ships XK/XV/XQ quantized to fp8 (plus a bf16 copy of XQ for
    the residual add), and Wq/Wk/Wv transposed and scaled by 16 in fp8
    (unscaled they'd sit in the subnormal range), Wo transposed in bf16.
  * The x16 weight scales cancel: exp(scale*x) uses scale 1/(256*16);
    V's guard columns hold -16 so PSUM accumulates -16*r, and
    Z = XQ + O * (-1/(16 r)) == XQ - attn.
  * exp also carries bias -ln(16), attenuating p~ = exp(s)/16 into
    e4m3's range (max |p| ~42 vs the 240 cap; scores are ~N(0,1) but the
    tail over 4M samples reaches ~6.5).
  * The attention term is only ~7% of Z's magnitude, so fp8 noise in the
    whole attention pipeline is diluted ~14x; the bf16 output projection
    sets the ~2e-3 floor.
  * Heads are processed in PAIRS (adjacent heads = adjacent n-columns),
    making most matmul frees and all drains 512 wide, halving fixed
    per-instruction costs. O-matmul free is 260 (odd frees like 258 run
    ~8x slow on the PE; 260 is even and full speed).
  * PSUM->SBUF drains are the co-bottleneck (DMA and GPSIMD have no PSUM
    port): interleaved between ScalarE and VectorE per group, ~78us each
    vs ~110us of PE work (PE sits at its activity-throttle bound).
  * The pipeline is woven at psum-group granularity: projections of pair
    i alongside scores of pair i-1 and O/output-projection of pair i-2,
    so producers finished a full step before their consumers issue.
  * dma_start costs ~600ns of serial descriptor-gen on the sync engine:
    inputs ship as one DMA per tensor and outputs two groups per DMA.
Measured: ~134us vs the 214us fp32r baseline (same structure), with
~13us of fixed engine preamble/postamble and a hard activity throttle
(PE util capped ~87%) setting the floor.
"""

import sys

if "/opt/trn_rl_repo" not in sys.path:
    sys.path.insert(0, "/opt/trn_rl_repo")

import numpy as np
import ml_dtypes

import concourse.tile as tile
from concourse import bacc, mybir
from concourse.bass_utils import run_bass_kernel_spmd

B_PER_CORE = 2
D = 512
N = 2048
H = 8
NH = N // H  # 256
PT = D // 128  # 4 partition tiles over D
HP = H // 2  # 4 head pairs
NP = 2 * NH  # 512 columns per head pair

F32 = mybir.dt.float32
F8 = mybir.dt.float8e4
BF16 = mybir.dt.bfloat16
DR = mybir.MatmulPerfMode.DoubleRow

NPF8 = ml_dtypes.float8_e4m3
NPBF16 = ml_dtypes.bfloat16

WS = 16.0  # host-side weight scale for Wq/Wk/Wv in fp8
EXP_SCALE = 1.0 / (WS * WS * np.sqrt(NH))  # = 1/4096
EXP_BIAS = float(-np.log(16.0))  # p~ = exp(s)/16
VG = 260  # per-head V row: 256 data + 2 guard (-16) + 2 pad

_NC_CACHE = None


def build_nc():
    nc = bacc.Bacc("TRN2", target_bir_lowering=False, debug=False)

    xq8 = nc.dram_tensor("xq8", [B_PER_CORE, D, N], F8, kind="ExternalInput").ap()
    xqb = nc.dram_tensor("xqb", [B_PER_CORE, D, N], BF16, kind="ExternalInput").ap()
    xk8 = nc.dram_tensor("xk8", [B_PER_CORE, D, N], F8, kind="ExternalInput").ap()
    xv8 = nc.dram_tensor("xv8", [B_PER_CORE, D, N], F8, kind="ExternalInput").ap()
    wq8 = nc.dram_tensor("wq8", [D, D], F8, kind="ExternalInput").ap()
    wk8 = nc.dram_tensor("wk8", [D, D], F8, kind="ExternalInput").ap()
    wv8 = nc.dram_tensor("wv8", [D, D], F8, kind="ExternalInput").ap()
    wob = nc.dram_tensor("wob", [D, D], BF16, kind="ExternalInput").ap()
    out = nc.dram_tensor("out", [B_PER_CORE, D, N], BF16, kind="ExternalOutput").ap()

    with tile.TileContext(nc) as tc:
        with (
            tc.tile_pool(name="wpool", bufs=1) as wpool,
            tc.tile_pool(name="xpool", bufs=1) as xpool,
            tc.tile_pool(name="qkpool", bufs=6) as qkpool,
            tc.tile_pool(name="vpool", bufs=4) as vpool,
            tc.tile_pool(name="ptpool", bufs=6) as ptpool,
            tc.tile_pool(name="zpool", bufs=3) as zpool,
            tc.tile_pool(name="opool", bufs=6) as opool,
            tc.tile_pool(name="rpool", bufs=8) as rpool,
            tc.tile_pool(name="ps1", bufs=4, space="PSUM") as ps1,
            tc.tile_pool(name="pso", bufs=2, space="PSUM") as pso,
        ):
            # Resident weights: [p, it, o] = W.T[it*128+p, o] (fp8 x16),
            # loaded in per-it chunks to shorten the first matmul's dep.
            w_sb = {
                "wq": wpool.tile([128, PT, D], F8, name="w_wq", tag="w_wq"),
                "wk": wpool.tile([128, PT, D], F8, name="w_wk", tag="w_wk"),
                "wv": wpool.tile([128, PT, D], F8, name="w_wv", tag="w_wv"),
                "wo": wpool.tile([128, PT, D], BF16, name="w_wo", tag="w_wo"),
            }
            w_dram = {"wq": wq8, "wk": wk8, "wv": wv8, "wo": wob}

            def load_w(name):
                src = w_dram[name].rearrange("(t p) o -> p t o", p=128)
                nc.sync.dma_start(out=w_sb[name], in_=src)

            x_b = {
                "xq8": [xq8[b].rearrange("(t p) n -> p t n", p=128) for b in range(B_PER_CORE)],
                "xqb": [xqb[b].rearrange("(t p) n -> p t n", p=128) for b in range(B_PER_CORE)],
                "xk8": [xk8[b].rearrange("(t p) n -> p t n", p=128) for b in range(B_PER_CORE)],
                "xv8": [xv8[b].rearrange("(t p) n -> p t n", p=128) for b in range(B_PER_CORE)],
            }
            x_dt = {"xq8": F8, "xqb": BF16, "xk8": F8, "xv8": F8}

            # Whole-batch input tiles, DMA'd in per-it chunks (256-512 KiB,
            # 2 KiB bursts) up front; per-pair slices view into these.
            x_sb = [
                {
                    nm: xpool.tile(
                        [128, PT, N], x_dt[nm], name=f"{nm}_b{b}", tag=f"{nm}_b{b}"
                    )
                    for nm in ("xq8", "xqb", "xk8", "xv8")
                }
                for b in range(B_PER_CORE)
            ]

            def load_x(b, nm):
                nc.sync.dma_start(out=x_sb[b][nm], in_=x_b[nm][b])

            steps = [(b, hp) for b in range(B_PER_CORE) for hp in range(HP)]
            NSTEP = len(steps)
            # per-step live tiles for the software pipeline
            st_qt = {}
            st_kt = {}
            st_v = {}
            st_pt = {}
            # (b, hp, z_pair) whose output projection hasn't been emitted yet
            pending_out = []

            # Output DMAs are paired (two dt groups per dma_start) to halve
            # the ~600ns/DMA sync-engine descriptor-gen cost.
            o_pair = {}

            def emit_outproj_group(b, hp, z_p, dt_, eng="act"):
                """One [128, 512] output-projection group (bf16)."""
                out_b = out[b].rearrange("(t p) n -> p t n", p=128)
                ns_ = slice(hp * NP, (hp + 1) * NP)
                ps = ps1.tile([128, D], F32, name="ps_p", tag="ps_p")
                for it in range(PT):
                    nc.tensor.matmul(
                        ps,
                        lhsT=w_sb["wo"][:, it, dt_ * 128 : (dt_ + 1) * 128],
                        rhs=z_p[:, it, :],
                        start=(it == 0),
                        stop=(it == PT - 1),
                    )
                if dt_ % 2 == 0:
                    o_pair[(b, hp)] = opool.tile([128, 2, D], BF16, name="o_sb", tag="o_sb")
                o_sb = o_pair[(b, hp)]
                if eng == "act":
                    nc.scalar.copy(out=o_sb[:, dt_ % 2, :], in_=ps)
                else:
                    nc.vector.tensor_copy(out=o_sb[:, dt_ % 2, :], in_=ps)
                if dt_ % 2 == 1:
                    nc.sync.dma_start(
                        out=out_b[:, dt_ - 1 : dt_ + 1, ns_],
                        in_=o_pair.pop((b, hp)),
                    )

            warm = wpool.tile([128, D], BF16, name="warm", tag="warm")
            bias_t = wpool.tile([128, 1], F32, name="bias_t", tag="bias_t")

            def xs(idx):
                b, hp = steps[idx]
                ns = slice(hp * NP, (hp + 1) * NP)
                return {nm: t[:, :, ns] for nm, t in x_sb[b].items()}

            # ps1 drains alternate between ScalarE and VectorE per group.
            dr_tog = [0]

            def drain(dst, src):
                dr_tog[0] ^= 1
                if dr_tog[0]:
                    nc.scalar.copy(out=dst, in_=src)
                else:
                    nc.vector.tensor_copy(out=dst, in_=src)

            def gen_s1s2(idx):
                """12 psum-groups: QT/KT (8) then V (4), all fp8 DR."""
                xt = xs(idx)
                st_qt[idx] = []
                st_kt[idx] = []
                for hl in range(2):
                    qt_h = qkpool.tile([128, 2, D], F8, name="qt_h", tag="qt_h")
                    kt_h = qkpool.tile([128, 2, D], F8, name="kt_h", tag="kt_h")
                    for dst, src, w in ((qt_h, xt["xq8"], "wq"), (kt_h, xt["xk8"], "wk")):
                        for jt in range(2):
                            nb = hl * NH + jt * 128
                            ps = ps1.tile([128, D], F32, name="ps_p", tag="ps_p")
                            for itp in range(2):
                                nc.tensor.matmul(
                                    ps,
                                    lhsT=src[:, 2 * itp : 2 * itp + 2, nb : nb + 128],
                                    rhs=w_sb[w][:, 2 * itp : 2 * itp + 2, :],
                                    start=(itp == 0),
                                    stop=(itp == 1),
                                    perf_mode=DR,
                                )
                            drain(dst[:, jt, :], ps)
                            yield
                    st_qt[idx].append(qt_h)
                    st_kt[idx].append(kt_h)
                v_p = vpool.tile([128, PT, 2 * VG], F8, name="v_p", tag="v_p")
                st_v[idx] = v_p
                guard = v_p.rearrange("p a (g c) -> p (a g) c", c=VG)[:, :, NH:VG]
                gw = VG - NH
                nc.scalar.activation(
                    out=guard,
                    in_=w_sb["wv"][:, 0, 0 : 8 * gw].rearrange("p (a c) -> p a c", c=gw),
                    func=mybir.ActivationFunctionType.Copy,
                    bias=-16.0,
                    scale=0.0,
                )
                for et in range(PT):
                    ps = ps1.tile([128, D], F32, name="ps_p", tag="ps_p")
                    for itp in range(2):
                        nc.tensor.matmul(
                            ps,
                            lhsT=w_sb["wv"][:, 2 * itp : 2 * itp + 2, et * 128 : (et + 1) * 128],
                            rhs=xt["xv8"][:, 2 * itp : 2 * itp + 2, :],
                            start=(itp == 0),
                            stop=(itp == 1),
                            perf_mode=DR,
                        )
                    dst = v_p[:, et, :].rearrange("p (g c) -> p g c", c=VG)[:, :, 0:NH]
                    drain(dst, ps.rearrange("p (g c) -> p g c", c=NH))
                    yield

            def gen_s3(idx):
                """4 psum-group slots: scores^T per et (1-bank) + exp."""
                qt, kt = st_qt.pop(idx), st_kt.pop(idx)
                st_pt[idx] = []
                for hl in range(2):
                    pt_h = ptpool.tile([128, PT, D], F8, name="pt_h", tag="pt_h")
                    for ep in range(2):
                        for i in range(2):
                            et = 2 * ep + i
                            ps = ps1.tile([128, D], F32, name="ps_s", tag="ps_p")
                            nc.tensor.matmul(
                                ps,
                                lhsT=kt[hl][:, 0:2, et * 128 : (et + 1) * 128],
                                rhs=qt[hl][:, 0:2, :],
                                start=True,
                                stop=True,
                                perf_mode=DR,
                            )
                            nc.scalar.activation(
                                out=pt_h[:, et, :],
                                in_=ps,
                                func=mybir.ActivationFunctionType.Exp,
                                scale=float(EXP_SCALE),
                                bias=bias_t,
                            )
                        yield
                    st_pt[idx].append(pt_h)

            def gen_s4(idx):
                """4 psum-groups: O + recip + STT; consumes one pending
                output-projection group after each."""
                b, hp = steps[idx]
                xt = xs(idx)
                pt = st_pt.pop(idx)
                v_p = st_v.pop(idx)
                z_p = zpool.tile([128, PT, NP], BF16, name="z_p", tag="z_p")
                for hl in range(2):
                    for dtp in range(2):
                        ps2 = pso.tile([128, 2, D], F32, name="ps_o", tag="ps_o")
                        for i in range(2):
                            dt_ = 2 * dtp + i
                            for etp in range(2):
                                nc.tensor.matmul(
                                    ps2[:, i, 0:VG],
                                    lhsT=pt[hl][:, 2 * etp : 2 * etp + 2, dt_ * 128 : (dt_ + 1) * 128],
                                    rhs=v_p[:, 2 * etp : 2 * etp + 2, hl * VG : (hl + 1) * VG],
                                    start=(etp == 0),
                                    stop=(etp == 1),
                                    perf_mode=DR,
                                )
                        recip = rpool.tile([128, 2], F32, name="recip", tag="recip")
                        nc.vector.reciprocal(recip, ps2[:, :, NH : NH + 1])
                        for i in range(2):
                            dt_ = 2 * dtp + i
                            nc.vector.scalar_tensor_tensor(
                                out=z_p[:, dt_, hl * NH : (hl + 1) * NH],
                                in0=ps2[:, i, 0:NH],
                                scalar=recip[:, i : i + 1],
                                in1=xt["xqb"][:, dt_, hl * NH : (hl + 1) * NH],
                                op0=mybir.AluOpType.mult,
                                op1=mybir.AluOpType.add,
                            )
                        if pending_out:
                            pb, php, pz, groups = pending_out[0]
                            # During the final step, keep these drains off
                            # VectorE: the last pair's recip/STT chain is the
                            # critical path and must not queue behind them.
                            eng = (
                                "act"
                                if idx == NSTEP - 1
                                else ("act" if dr_tog[0] else "dve")
                            )
                            emit_outproj_group(pb, php, pz, groups.pop(0), eng=eng)
                            if not groups:
                                pending_out.pop(0)
                        yield
                pending_out.append((b, hp, z_p, list(range(PT))))

            # Software pipeline, woven at psum-group granularity: stage s1s2
            # of pair i runs alongside scores of pair i-1 and O of pair i-2,
            # so consecutive PE groups hit different PSUM pools and their
            # producers finished a full step earlier.
            WEAVE = ["a", "a", "a", "b", "c"] * 4
            for it_ in range(NSTEP + 2):
                if it_ == 0:
                    nc.vector.memset(warm, 0.0)
                    nc.scalar.activation(
                        out=bias_t,
                        in_=warm[:, 0:2].bitcast(F32),
                        func=mybir.ActivationFunctionType.Copy,
                        bias=EXP_BIAS,
                        scale=0.0,
                    )
                    # PE warmup on zeros: flip the HAM clock gate to 8/8 and
                    # ramp the p-state during the initial DMA window.
                    for _ in range(5):
                        ps_w = ps1.tile([128, D], F32, name="ps_p", tag="ps_p")
                        for _ in range(4):
                            nc.tensor.matmul(
                                ps_w, lhsT=warm[:, 0:128], rhs=warm, start=True, stop=True
                            )
                    # Startup: one DMA per tensor — each dma_start costs
                    # ~600ns of serial descriptor-gen on the sync engine, so
                    # fewer/bigger beats chunked (the first matmul needs the
                    # whole tensor anyway: contraction spans all of D).
                    load_w("wq")
                    load_x(0, "xq8")
                    load_w("wk")
                    load_x(0, "xk8")
                    load_w("wv")
                    load_x(0, "xv8")
                    load_x(0, "xqb")
                    load_w("wo")
                    load_x(1, "xq8")
                    load_x(1, "xk8")
                    load_x(1, "xv8")
                    load_x(1, "xqb")
                gens = {}
                if it_ < NSTEP:
                    gens["a"] = gen_s1s2(it_)
                if 1 <= it_ <= NSTEP:
                    gens["b"] = gen_s3(it_ - 1)
                if 2 <= it_ <= NSTEP + 1:
                    gens["c"] = gen_s4(it_ - 2)
                for s in WEAVE:
                    if s in gens:
                        next(gens[s], None)
                for g in gens.values():
                    for _ in g:
                        pass

            for pb, php, pz, groups in pending_out:
                for g in groups:
                    emit_outproj_group(pb, php, pz, g, eng="act" if g % 2 == 0 else "dve")

    nc.compile()
    return nc


def _get_nc():
    global _NC_CACHE
    if _NC_CACHE is None:
        _NC_CACHE = build_nc()
    return _NC_CACHE


def _shard_inputs(inputs):
    xq = np.ascontiguousarray(np.asarray(inputs["X_Query"], dtype=np.float32))
    xk = np.ascontiguousarray(np.asarray(inputs["X_Key"], dtype=np.float32))
    xv = np.ascontiguousarray(np.asarray(inputs["X_Value"], dtype=np.float32))
    xq8 = xq.astype(NPF8)
    xqb = xq.astype(NPBF16)
    xk8 = xk.astype(NPF8)
    xv8 = xv.astype(NPF8)
    weights = {
        "wq8": np.ascontiguousarray(
            (np.asarray(inputs["W_q"], dtype=np.float32).T * WS).astype(NPF8)
        ),
        "wk8": np.ascontiguousarray(
            (np.asarray(inputs["W_k"], dtype=np.float32).T * WS).astype(NPF8)
        ),
        "wv8": np.ascontiguousarray(
            (np.asarray(inputs["W_v"], dtype=np.float32).T * WS).astype(NPF8)
        ),
        "wob": np.ascontiguousarray(
            np.asarray(inputs["W_o"], dtype=np.float32).T.astype(NPBF16)
        ),
    }
    in_maps = []
    for c in range(8):
        sl = slice(c * B_PER_CORE, (c + 1) * B_PER_CORE)
        in_maps.append(
            {
                "xq8": xq8[sl],
                "xqb": xqb[sl],
                "xk8": xk8[sl],
                "xv8": xv8[sl],
                **weights,
            }
        )
    return in_maps


def run_sharded(inputs, **kwargs):
    """Run on all 8 cores; returns (full_output, BassKernelResults)."""
    nc = _get_nc()
    in_maps = _shard_inputs(inputs)
    res = run_bass_kernel_spmd(nc, in_maps, core_ids=list(range(8)), **kwargs)
    full = np.concatenate(
        [np.asarray(r["out"]).astype(np.float32) for r in res.results], axis=0
    )
    return full, res


def kernel(**inputs):
    full, _ = run_sharded(inputs)
    return full
